# revision 37
# baseline (speedup 1.0000x reference)
"""HQQ 4-bit quantized linear layer on 8 Trainium2 NeuronCores.

Reference computation:
    W_r = concat([W_q >> 4, W_q & 0xF], axis=0).astype(f32)    # [64, 704512]
    W   = ((W_r - zero) * scale).reshape(11008, 4096)          # [out, in]
    out = x @ W.T + bias                                        # [4, 2048, 11008]

Group structure: group j = r*4096 + k (r in [0,172), k in [0,4096)) supplies
output feature o = i*172 + r (element i in [0,64) of the group) at input
feature k.  So for a fixed o, every k belongs to a different group, and
W[o, k] = (nib[i, j] - zero[j]) * scale[j] with i = o//172, j = (o%172)*4096+k.

Sharding (column-parallel over output features, SPMD-uniform):
  core c owns W_q byte-rows [4c, 4c+4).  Byte-row b holds the high nibble of
  i=b and the low nibble of i=b+32, so core c produces output features
  o in {(4c+ib)*172 + r} (high) and {(32+4c+ib)*172 + r} (low), ib in [0,4),
  r in [0,172): 1376 contiguous-after-gather features per core.  Every core
  runs the identical program; x / scale / zero are replicated.

Host-side prep: x is transposed and cast to bf16; the per-core weight shard
is dequantized to bf16 [K, 1376] (k-major, exactly the affine the reference
applies, in f32 then rounded once to bf16) and the trailing FP8_KT k-tiles
are additionally pre-cast to fp8e4m3 in the DoubleRow pair-interleaved
layout.  The device streams weights straight into resident SBUF (no on-
device dequant dependency chain, so the prologue is DMA-paced and the PE
starts within a few us).

Device kernel (per core):
  - stream the 32 w k-tiles [128, 1376] bf16 + 3 fp8 pair tiles into
    resident SBUF.
  - stream bf16 x^T tiles [k=128, tokens] (one DMA per 512-token superstep);
    per 128-token sub, matmul-accumulate over k into PSUM (tokens on psum
    partitions): 26 bf16 matmuls + 3 fp8 DoubleRow pair matmuls (2 k-tiles
    per instruction at the bf16 column rate) per 512-out chunk; x fp8 copies
    are cast on ScalarE per superstep.
  - the first superstep runs kt-major in sub-pairs so matmuls consume each
    weight tile as its DMA lands.
  - drain PSUM + bias (broadcast tile) on VectorE, DMA out f32 per chunk.
"""

import os
import sys

for _p in ("/opt/trn_rl_repo",):
    if os.path.isdir(_p) and _p not in sys.path:
        sys.path.insert(0, _p)

import numpy as np
import ml_dtypes

P = 128
IN_F = 4096
OUT_F = 11008
GROUP = 64
R_FULL = 172          # OUT_F // GROUP
IB_FULL = 4           # W_q byte rows per core
N_CORES = 8
NTOK_FULL = 8192      # 4 * 2048

FP8_KT = 6            # trailing k-tiles computed in fp8 DoubleRow pairs


def _chunks(n, maxc=512):
    out = []
    off = 0
    while off < n:
        sz = min(maxc, n - off)
        out.append((off, sz))
        off += sz
    return out


def build_program(KT=32, NSUP=16, SUP=512, IB=IB_FULL, R=R_FULL,
                  num_devices=N_CORES, fp8_kt=FP8_KT, ktmajor_ns=1):
    """Build the SPMD bass program. Returns the compiled Bacc object.

    KT: number of 128-wide k tiles (K = 128*KT)
    NSUP: number of token supersteps;  SUP: tokens per superstep (mult of 128)
    IB: W_q byte rows per core;  R: group minor dim (o = i*R + r)
    fp8_kt: number of trailing k-tiles run as fp8 DoubleRow pairs (even)
    """
    import concourse.bacc as bacc
    import concourse.bass as bass
    import concourse.mybir as mybir
    import concourse.tile as tile
    from concourse.alu_op_type import AluOpType

    f32 = mybir.dt.float32
    bf16 = mybir.dt.bfloat16
    fp8 = mybir.dt.float8e4

    assert fp8_kt % 2 == 0
    bf_kt = KT - fp8_kt

    K = KT * P
    NTOK = NSUP * SUP
    NSUB = SUP // P
    OHALF = IB * R
    OFULL = 2 * OHALF
    CHUNKS = _chunks(OFULL)

    nc = bacc.Bacc(
        "TRN2", target_bir_lowering=False, debug=False, num_devices=num_devices
    )

    xt = nc.dram_tensor("xt", [K, NTOK], bf16, kind="ExternalInput")
    wt = nc.dram_tensor("wt", [K, OFULL], bf16, kind="ExternalInput")
    w8t = (nc.dram_tensor("w8t", [fp8_kt // 2, P, 2 * OFULL], fp8,
                          kind="ExternalInput") if fp8_kt else None)
    bias = nc.dram_tensor("bias", [OFULL], f32, kind="ExternalInput")
    out = nc.dram_tensor("out", [NTOK, OFULL], f32, kind="ExternalOutput")

    with tile.TileContext(nc) as tc:
        with (
            tc.tile_pool(name="cst", bufs=1) as cst,
            tc.tile_pool(name="wres", bufs=1) as wres,
            tc.tile_pool(name="xb", bufs=2) as xbp,
            tc.tile_pool(name="psum", bufs=2, space="PSUM") as pp,
            tc.tile_pool(name="psum0", bufs=3, space="PSUM") as pp0,
            tc.tile_pool(name="psumw", bufs=1, space="PSUM") as ppw,
            tc.tile_pool(name="outp", bufs=2) as op,
        ):
            def load_x(tok0, nsplit=1, xb=None, splits=None):
                if xb is None:
                    xb = xbp.tile([P, KT, SUP], bf16, tag="xb")
                step = KT // nsplit
                for si in (range(nsplit) if splits is None else splits):
                    kt0 = si * step
                    src = bass.AP(
                        xt, kt0 * P * NTOK + tok0,
                        [[NTOK, P], [P * NTOK, step], [1, SUP]],
                    )
                    nc.sync.dma_start(
                        out=xb[:, kt0:kt0 + step, :], in_=src
                    )
                return xb

            # PE warm-up: dummy matmuls on scratch SBUF keep the HAM activity
            # window busy from t~0 so the first real matmuls run at full clock
            warm = cst.tile([P, 640], bf16)
            nc.vector.memset(warm[:], 0)
            psw = ppw.tile([P, 512], f32, tag="psw", name="psw")
            NWARM = 44
            for wi in range(NWARM):
                nc.tensor.matmul(
                    psw[:, :128], warm[:, :P], warm[:, P:128 + P],
                    start=(wi == 0), stop=(wi == NWARM - 1),
                    skip_group_check=True,
                )

            # --- stream the weight shard into resident SBUF ---
            # bf16 tiles only for the bf16 k-range; the fp8 k-tiles live
            # exclusively as w8_res pair tiles
            w_res = [
                wres.tile([P, OFULL], bf16, tag=f"w{kt}", name=f"w{kt}")
                for kt in range(bf_kt)
            ]
            w8_res = [
                wres.tile([P, 2, OFULL], fp8, tag=f"w8_{j}", name=f"w8_{j}")
                for j in range(fp8_kt // 2)
            ]
            # issue order matches first-superstep consumption: x split 0 and
            # the first w tiles first, remaining x splits interleaved just
            # ahead of when their k-tiles are reached, fp8 pair tiles last
            def load_w(kts):
                for kt in kts:
                    nc.sync.dma_start(
                        out=w_res[kt][:], in_=wt[kt * P:(kt + 1) * P, :]
                    )

            xb0 = load_x(0, nsplit=8, splits=[0])
            load_w(range(0, 4))
            load_x(0, nsplit=8, xb=xb0, splits=[1])
            load_w(range(4, 8))
            load_x(0, nsplit=8, xb=xb0, splits=[2])
            load_w(range(8, 12))
            load_x(0, nsplit=8, xb=xb0, splits=[3])
            load_w(range(12, 16))
            load_x(0, nsplit=8, xb=xb0, splits=[4])
            load_w(range(16, 20))
            load_x(0, nsplit=8, xb=xb0, splits=[5])
            load_w(range(20, 24))
            load_x(0, nsplit=8, xb=xb0, splits=[6])
            load_w(range(24, bf_kt))
            load_x(0, nsplit=8, xb=xb0, splits=[7])
            for j in range(fp8_kt // 2):
                nc.sync.dma_start(
                    out=w8_res[j][:].rearrange("p a b -> p (a b)"),
                    in_=w8t[j, :, :],
                )

            # bias broadcast to [128, OFULL] via partition-step-0 DMA read
            bias_b = cst.tile([P, OFULL], f32)
            bias_bcast_src = bass.AP(bias, 0, [[0, P], [1, OFULL]])
            nc.sync.dma_start(out=bias_b[:], in_=bias_bcast_src)

            # --- main GEMM loop ---
            def drain(sub, tok0, ps):
                ot = op.tile([P, OFULL], f32, tag="ot")
                row0 = tok0 + sub * P
                for ci, (off, sz) in enumerate(CHUNKS):
                    nc.vector.tensor_tensor(
                        out=ot[:, off:off + sz], in0=ps[ci][:],
                        in1=bias_b[:, off:off + sz], op=AluOpType.add,
                    )
                    nc.sync.dma_start(
                        out=out[row0:row0 + P, off:off + sz],
                        in_=ot[:, off:off + sz],
                    )
                    # zero the bank after reading: the next generation then
                    # accumulates with start=False, skipping the ~512-cycle
                    # bank clear that start=True costs the PE per sub
                    nc.vector.memset(ps[ci][:], 0.0)

            def cast_x8(xb):
                if not fp8_kt:
                    return None
                x8 = xbp.tile([P, fp8_kt // 2, 2, SUP], fp8, tag="x8")
                for j in range(fp8_kt // 2):
                    for sl in range(2):
                        nc.scalar.copy(
                            out=x8[:, j, sl, :],
                            in_=xb[:, bf_kt + 2 * j + sl, :],
                        )
                return x8

            def matmuls(ps, xb, x8, sub, chunk_major=False, dr_first=False):
                ts = slice(sub * P, (sub + 1) * P)
                # first chunk per unit is the narrow 352-wide one: the
                # per-sub-boundary stall costs ~one fill of the FIRST matmul,
                # so make that matmul the cheapest
                chunk_iter = (
                    [[ci] for ci in range(len(CHUNKS))] if chunk_major
                    else [[len(CHUNKS) - 1] + list(range(len(CHUNKS) - 1))]
                )
                # alternating DR position across subs keeps consecutive subs'
                # DoubleRow blocks adjacent, halving DR<->bf16 mode switches
                bf_units = [("bf", kt) for kt in range(bf_kt)]
                f8_units = [("f8", j) for j in range(fp8_kt // 2)]
                units = (f8_units + bf_units) if dr_first                     else (bf_units + f8_units)
                last = len(units) - 1
                for cis in chunk_iter:
                    for ui, (kind, idx) in enumerate(units):
                        if kind == "bf":
                            lhsT = xb[:, idx, ts]
                            for ci in cis:
                                off, sz = CHUNKS[ci]
                                nc.tensor.matmul(
                                    ps[ci][:],
                                    lhsT,
                                    w_res[idx][:, off:off + sz],
                                    start=False,
                                    stop=(ui == last),
                                    skip_group_check=True,
                                )
                        else:
                            lhsT8 = x8[:, idx, :, ts]
                            for ci in cis:
                                off, sz = CHUNKS[ci]
                                nc.tensor.matmul(
                                    ps[ci][:],
                                    lhsT8,
                                    w8_res[idx][:, :, off:off + sz],
                                    start=False,
                                    stop=(ui == last),
                                    skip_group_check=True,
                                    perf_mode=mybir.MatmulPerfMode.DoubleRow,
                                )

            for ns in range(NSUP):
                tok0 = ns * SUP
                xb = xb0 if ns == 0 else load_x(tok0)
                x8 = cast_x8(xb)
                if ns < ktmajor_ns:
                    # kt-major order in sub-pairs: consume each weight tile
                    # across 2 subs x 3 chunks as soon as its DMA lands
                    for sub0 in range(0, NSUB, 2):
                        subs = [sub0, sub0 + 1][:NSUB - sub0]
                        ps_all = [
                            [
                                (pp0 if ci == 0 else pp).tile(
                                    [P, sz], f32, tag=f"ps{ci}",
                                    name=f"ps{ci}")
                                for ci, (off, sz) in enumerate(CHUNKS)
                            ]
                            for _ in subs
                        ]
                        for kt in range(bf_kt):
                            for si, sub in enumerate(subs):
                                lhsT = xb[:, kt, sub * P:(sub + 1) * P]
                                for ci, (off, sz) in enumerate(CHUNKS):
                                    nc.tensor.matmul(
                                        ps_all[si][ci][:],
                                        lhsT,
                                        w_res[kt][:, off:off + sz],
                                        start=(kt == 0),
                                        stop=(fp8_kt == 0
                                              and kt == bf_kt - 1),
                                    )
                        for j in range(fp8_kt // 2):
                            for si, sub in enumerate(subs):
                                lhsT8 = x8[:, j, :, sub * P:(sub + 1) * P]
                                for ci, (off, sz) in enumerate(CHUNKS):
                                    nc.tensor.matmul(
                                        ps_all[si][ci][:],
                                        lhsT8,
                                        w8_res[j][:, :, off:off + sz],
                                        start=False,
                                        stop=(j == fp8_kt // 2 - 1),
                                        perf_mode=(
                                            mybir.MatmulPerfMode.DoubleRow
                                        ),
                                    )
                        for si, sub in enumerate(subs):
                            drain(sub, tok0, ps_all[si])
                    continue
                for sub in range(NSUB):
                    ps = [
                        (pp0 if ci == 0 else pp).tile(
                            [P, sz], f32, tag=f"ps{ci}", name=f"ps{ci}")
                        for ci, (off, sz) in enumerate(CHUNKS)
                    ]
                    # chunk-major on the final sub so chunk drains + output
                    # DMAs overlap the remaining chunks' matmuls (shorter tail)
                    matmuls(ps, xb, x8, sub,
                            chunk_major=(ns == NSUP - 1 and sub == NSUB - 1),
                            dr_first=((ns * NSUB + sub) % 2 == 1))
                    drain(sub, tok0, ps)

    nc.compile()
    return nc


_PROG_CACHE = {}


def _get_program():
    key = "full"
    if key not in _PROG_CACHE:
        _PROG_CACHE[key] = build_program()
    return _PROG_CACHE[key]


def shard_inputs(x, W_q, scale, zero, bias, fp8_kt=FP8_KT, KT=32):
    """Host-side sharding / layout / dtype prep."""
    x = np.asarray(x, dtype=np.float32)
    W_q = np.asarray(W_q)
    scale = np.asarray(scale, dtype=np.float32)
    zero = np.asarray(zero, dtype=np.float32)
    bias = np.asarray(bias, dtype=np.float32)
    bf_kt = KT - fp8_kt

    ntok = x.shape[0] * x.shape[1]
    xt = np.ascontiguousarray(
        x.reshape(ntok, IN_F).T.astype(ml_dtypes.bfloat16)
    )                                                               # [K, NTOK]
    scale_t = np.ascontiguousarray(scale.reshape(R_FULL, IN_F).T)   # [K, R]
    zero_t = np.ascontiguousarray(zero.reshape(R_FULL, IN_F).T)     # [K, R]
    wq_u8 = W_q.astype(np.uint8)                                    # values < 256
    bias_rs = bias.reshape(GROUP, R_FULL)                           # [i, r]

    # [K, OHALF] ib-replicated zero/scale, shared by all cores
    zr = np.ascontiguousarray(
        zero_t[:, None, :].repeat(IB_FULL, 1)).reshape(IN_F, -1)
    sc = np.ascontiguousarray(
        scale_t[:, None, :].repeat(IB_FULL, 1)).reshape(IN_F, -1)
    OH = IB_FULL * R_FULL

    in_maps = []
    for c in range(N_CORES):
        rows = wq_u8[IB_FULL * c: IB_FULL * (c + 1)]                # [4, 704512]
        # [ib, r, k] -> [k, ib, r] -> [K, OHALF]
        hi = np.ascontiguousarray(
            (rows >> 4).reshape(IB_FULL, R_FULL, IN_F).transpose(2, 0, 1)
        ).reshape(IN_F, IB_FULL * R_FULL).astype(np.float32)
        lo = np.ascontiguousarray(
            (rows & 0xF).reshape(IB_FULL, R_FULL, IN_F).transpose(2, 0, 1)
        ).reshape(IN_F, IB_FULL * R_FULL).astype(np.float32)
        wf = np.empty((IN_F, 2 * IB_FULL * R_FULL), dtype=np.float32)
        wf[:, :OH] = (hi - zr) * sc
        wf[:, OH:] = (lo - zr) * sc
        wt = wf.astype(ml_dtypes.bfloat16)

        bias_c = np.concatenate(
            [
                bias_rs[IB_FULL * c: IB_FULL * (c + 1)].ravel(),
                bias_rs[32 + IB_FULL * c: 32 + IB_FULL * (c + 1)].ravel(),
            ]
        )
        m = {"xt": xt, "wt": wt, "bias": bias_c}
        if fp8_kt:
            # fp8 pair tiles cast straight from the f32 dequant (skips the
            # intermediate bf16 rounding of the bf16 path)
            w8 = np.empty((fp8_kt // 2, P, 2, 2 * OH),
                          dtype=ml_dtypes.float8_e4m3)
            for j in range(fp8_kt // 2):
                for sl in range(2):
                    kt = bf_kt + 2 * j + sl
                    w8[j, :, sl, :] = wt[kt * P:(kt + 1) * P, :].astype(
                        ml_dtypes.float8_e4m3)
            m["w8t"] = w8.reshape(fp8_kt // 2, P, 4 * OH)
        in_maps.append(m)
    return in_maps


def gather_output(results, ntok=NTOK_FULL):
    out = np.empty((ntok, OUT_F), dtype=np.float32)
    ohalf = IB_FULL * R_FULL
    for c in range(N_CORES):
        res = results[c]["out"]
        lo = IB_FULL * c * R_FULL
        out[:, lo: lo + ohalf] = res[:, :ohalf]
        lo = (32 + IB_FULL * c) * R_FULL
        out[:, lo: lo + ohalf] = res[:, ohalf:]
    return out


def kernel(x, W_q, scale, zero, bias):
    from concourse.bass_utils import run_bass_kernel_spmd

    x = np.asarray(x)
    b, s, _ = x.shape
    nc = _get_program()
    in_maps = shard_inputs(x, W_q, scale, zero, bias)
    res = run_bass_kernel_spmd(nc, in_maps, list(range(N_CORES)))
    out = gather_output(res.results)
    return out.reshape(b, s, OUT_F)


# revision 39
# speedup vs baseline: 1.0012x; 1.0012x over previous
"""HQQ 4-bit quantized linear layer on 8 Trainium2 NeuronCores.

Reference computation:
    W_r = concat([W_q >> 4, W_q & 0xF], axis=0).astype(f32)    # [64, 704512]
    W   = ((W_r - zero) * scale).reshape(11008, 4096)          # [out, in]
    out = x @ W.T + bias                                        # [4, 2048, 11008]

Group structure: group j = r*4096 + k (r in [0,172), k in [0,4096)) supplies
output feature o = i*172 + r (element i in [0,64) of the group) at input
feature k.  So for a fixed o, every k belongs to a different group, and
W[o, k] = (nib[i, j] - zero[j]) * scale[j] with i = o//172, j = (o%172)*4096+k.

Sharding (column-parallel over output features, SPMD-uniform):
  core c owns W_q byte-rows [4c, 4c+4).  Byte-row b holds the high nibble of
  i=b and the low nibble of i=b+32, so core c produces output features
  o in {(4c+ib)*172 + r} (high) and {(32+4c+ib)*172 + r} (low), ib in [0,4),
  r in [0,172): 1376 contiguous-after-gather features per core.  Every core
  runs the identical program; x / scale / zero are replicated.

Host-side prep: x is transposed and cast to bf16; the per-core weight shard
is dequantized to bf16 [K, 1376] (k-major, exactly the affine the reference
applies, in f32 then rounded once to bf16) and the trailing FP8_KT k-tiles
are additionally pre-cast to fp8e4m3 in the DoubleRow pair-interleaved
layout.  The device streams weights straight into resident SBUF (no on-
device dequant dependency chain, so the prologue is DMA-paced and the PE
starts within a few us).

Device kernel (per core):
  - stream the 32 w k-tiles [128, 1376] bf16 + 3 fp8 pair tiles into
    resident SBUF.
  - stream bf16 x^T tiles [k=128, tokens] (one DMA per 512-token superstep);
    per 128-token sub, matmul-accumulate over k into PSUM (tokens on psum
    partitions): 26 bf16 matmuls + 3 fp8 DoubleRow pair matmuls (2 k-tiles
    per instruction at the bf16 column rate) per 512-out chunk; x fp8 copies
    are cast on ScalarE per superstep.
  - the first superstep runs kt-major in sub-pairs so matmuls consume each
    weight tile as its DMA lands.
  - drain PSUM + bias (broadcast tile) on VectorE, DMA out f32 per chunk.
"""

import os
import sys

for _p in ("/opt/trn_rl_repo",):
    if os.path.isdir(_p) and _p not in sys.path:
        sys.path.insert(0, _p)

import numpy as np
import ml_dtypes

P = 128
IN_F = 4096
OUT_F = 11008
GROUP = 64
R_FULL = 172          # OUT_F // GROUP
IB_FULL = 4           # W_q byte rows per core
N_CORES = 8
NTOK_FULL = 8192      # 4 * 2048

FP8_KT = 6            # trailing k-tiles computed in fp8 DoubleRow pairs


def _chunks(n, maxc=512):
    out = []
    off = 0
    while off < n:
        sz = min(maxc, n - off)
        out.append((off, sz))
        off += sz
    return out


def build_program(KT=32, NSUP=16, SUP=512, IB=IB_FULL, R=R_FULL,
                  num_devices=N_CORES, fp8_kt=FP8_KT, ktmajor_ns=1):
    """Build the SPMD bass program. Returns the compiled Bacc object.

    KT: number of 128-wide k tiles (K = 128*KT)
    NSUP: number of token supersteps;  SUP: tokens per superstep (mult of 128)
    IB: W_q byte rows per core;  R: group minor dim (o = i*R + r)
    fp8_kt: number of trailing k-tiles run as fp8 DoubleRow pairs (even)
    """
    import concourse.bacc as bacc
    import concourse.bass as bass
    import concourse.mybir as mybir
    import concourse.tile as tile
    from concourse.alu_op_type import AluOpType

    f32 = mybir.dt.float32
    bf16 = mybir.dt.bfloat16
    fp8 = mybir.dt.float8e4

    assert fp8_kt % 2 == 0
    bf_kt = KT - fp8_kt

    K = KT * P
    NTOK = NSUP * SUP
    NSUB = SUP // P
    OHALF = IB * R
    OFULL = 2 * OHALF
    CHUNKS = _chunks(OFULL)

    nc = bacc.Bacc(
        "TRN2", target_bir_lowering=False, debug=False, num_devices=num_devices
    )

    xt = nc.dram_tensor("xt", [K, NTOK], bf16, kind="ExternalInput")
    wt = nc.dram_tensor("wt", [K, OFULL], bf16, kind="ExternalInput")
    w8t = (nc.dram_tensor("w8t", [fp8_kt // 2, P, 2 * OFULL], fp8,
                          kind="ExternalInput") if fp8_kt else None)
    bias = nc.dram_tensor("bias", [OFULL], f32, kind="ExternalInput")
    out = nc.dram_tensor("out", [NTOK, OFULL], f32, kind="ExternalOutput")

    with tile.TileContext(nc) as tc:
        with (
            tc.tile_pool(name="cst", bufs=1) as cst,
            tc.tile_pool(name="wres", bufs=1) as wres,
            tc.tile_pool(name="xb", bufs=2) as xbp,
            tc.tile_pool(name="psum", bufs=2, space="PSUM") as pp,
            tc.tile_pool(name="psum0", bufs=3, space="PSUM") as pp0,
            tc.tile_pool(name="psumw", bufs=1, space="PSUM") as ppw,
            tc.tile_pool(name="outp", bufs=2) as op,
        ):
            def load_x(tok0, nsplit=1, xb=None, splits=None):
                if xb is None:
                    xb = xbp.tile([P, KT, SUP], bf16, tag="xb")
                step = KT // nsplit
                for si in (range(nsplit) if splits is None else splits):
                    kt0 = si * step
                    src = bass.AP(
                        xt, kt0 * P * NTOK + tok0,
                        [[NTOK, P], [P * NTOK, step], [1, SUP]],
                    )
                    nc.sync.dma_start(
                        out=xb[:, kt0:kt0 + step, :], in_=src
                    )
                return xb

            # PE warm-up: dummy matmuls on scratch SBUF keep the HAM activity
            # window busy from t~0 so the first real matmuls run at full clock
            warm = cst.tile([P, 640], bf16)
            nc.vector.memset(warm[:], 0)
            psw = ppw.tile([P, 512], f32, tag="psw", name="psw")
            NWARM = 32
            for wi in range(NWARM):
                nc.tensor.matmul(
                    psw[:, :128], warm[:, :P], warm[:, P:128 + P],
                    start=(wi == 0), stop=(wi == NWARM - 1),
                    skip_group_check=True,
                )

            # --- stream the weight shard into resident SBUF ---
            # bf16 tiles only for the bf16 k-range; the fp8 k-tiles live
            # exclusively as w8_res pair tiles
            w_res = [
                wres.tile([P, OFULL], bf16, tag=f"w{kt}", name=f"w{kt}")
                for kt in range(bf_kt)
            ]
            w8_res = [
                wres.tile([P, 2, OFULL], fp8, tag=f"w8_{j}", name=f"w8_{j}")
                for j in range(fp8_kt // 2)
            ]
            # issue order matches first-superstep consumption: x split 0 and
            # the first w tiles first, remaining x splits interleaved just
            # ahead of when their k-tiles are reached, fp8 pair tiles last
            def load_w(kts):
                for kt in kts:
                    nc.sync.dma_start(
                        out=w_res[kt][:], in_=wt[kt * P:(kt + 1) * P, :]
                    )

            xb0 = load_x(0, nsplit=8, splits=[0])
            load_w(range(0, 4))
            load_x(0, nsplit=8, xb=xb0, splits=[1])
            load_w(range(4, 8))
            load_x(0, nsplit=8, xb=xb0, splits=[2])
            load_w(range(8, 12))
            load_x(0, nsplit=8, xb=xb0, splits=[3])
            load_w(range(12, 16))
            load_x(0, nsplit=8, xb=xb0, splits=[4])
            load_w(range(16, 20))
            load_x(0, nsplit=8, xb=xb0, splits=[5])
            load_w(range(20, 24))
            load_x(0, nsplit=8, xb=xb0, splits=[6])
            load_w(range(24, bf_kt))
            load_x(0, nsplit=8, xb=xb0, splits=[7])
            for j in range(fp8_kt // 2):
                nc.sync.dma_start(
                    out=w8_res[j][:].rearrange("p a b -> p (a b)"),
                    in_=w8t[j, :, :],
                )

            # bias broadcast to [128, OFULL] via partition-step-0 DMA read
            bias_b = cst.tile([P, OFULL], f32)
            bias_bcast_src = bass.AP(bias, 0, [[0, P], [1, OFULL]])
            nc.sync.dma_start(out=bias_b[:], in_=bias_bcast_src)

            # --- main GEMM loop ---
            def drain(sub, tok0, ps):
                ot = op.tile([P, OFULL], f32, tag="ot")
                row0 = tok0 + sub * P
                for ci, (off, sz) in enumerate(CHUNKS):
                    nc.vector.tensor_tensor(
                        out=ot[:, off:off + sz], in0=ps[ci][:],
                        in1=bias_b[:, off:off + sz], op=AluOpType.add,
                    )
                    # out-DMAs ride the idle GpSimd queue so they never
                    # queue behind the 4MB x superstep loads on the sync queue
                    nc.gpsimd.dma_start(
                        out=out[row0:row0 + P, off:off + sz],
                        in_=ot[:, off:off + sz],
                    )

            def cast_x8(xb):
                if not fp8_kt:
                    return None
                x8 = xbp.tile([P, fp8_kt // 2, 2, SUP], fp8, tag="x8")
                for j in range(fp8_kt // 2):
                    for sl in range(2):
                        nc.scalar.copy(
                            out=x8[:, j, sl, :],
                            in_=xb[:, bf_kt + 2 * j + sl, :],
                        )
                return x8

            def matmuls(ps, xb, x8, sub, chunk_major=False, dr_first=False):
                ts = slice(sub * P, (sub + 1) * P)
                # first chunk per unit is the narrow 352-wide one: the
                # per-sub-boundary stall costs ~one fill of the FIRST matmul,
                # so make that matmul the cheapest
                chunk_iter = (
                    [[ci] for ci in range(len(CHUNKS))] if chunk_major
                    else [[len(CHUNKS) - 1] + list(range(len(CHUNKS) - 1))]
                )
                # alternating DR position across subs keeps consecutive subs'
                # DoubleRow blocks adjacent, halving DR<->bf16 mode switches
                bf_units = [("bf", kt) for kt in range(bf_kt)]
                f8_units = [("f8", j) for j in range(fp8_kt // 2)]
                units = (f8_units + bf_units) if dr_first                     else (bf_units + f8_units)
                last = len(units) - 1
                for cis in chunk_iter:
                    for ui, (kind, idx) in enumerate(units):
                        if kind == "bf":
                            lhsT = xb[:, idx, ts]
                            for ci in cis:
                                off, sz = CHUNKS[ci]
                                nc.tensor.matmul(
                                    ps[ci][:],
                                    lhsT,
                                    w_res[idx][:, off:off + sz],
                                    start=(ui == 0),
                                    stop=(ui == last),
                                )
                        else:
                            lhsT8 = x8[:, idx, :, ts]
                            for ci in cis:
                                off, sz = CHUNKS[ci]
                                nc.tensor.matmul(
                                    ps[ci][:],
                                    lhsT8,
                                    w8_res[idx][:, :, off:off + sz],
                                    start=(ui == 0),
                                    stop=(ui == last),
                                    perf_mode=mybir.MatmulPerfMode.DoubleRow,
                                )

            for ns in range(NSUP):
                tok0 = ns * SUP
                xb = xb0 if ns == 0 else load_x(tok0)
                x8 = cast_x8(xb)
                if ns < ktmajor_ns:
                    # kt-major order in sub-pairs: consume each weight tile
                    # across 2 subs x 3 chunks as soon as its DMA lands
                    for sub0 in range(0, NSUB, 2):
                        subs = [sub0, sub0 + 1][:NSUB - sub0]
                        ps_all = [
                            [
                                (pp0 if ci == 0 else pp).tile(
                                    [P, sz], f32, tag=f"ps{ci}",
                                    name=f"ps{ci}")
                                for ci, (off, sz) in enumerate(CHUNKS)
                            ]
                            for _ in subs
                        ]
                        for kt in range(bf_kt):
                            for si, sub in enumerate(subs):
                                lhsT = xb[:, kt, sub * P:(sub + 1) * P]
                                for ci, (off, sz) in enumerate(CHUNKS):
                                    nc.tensor.matmul(
                                        ps_all[si][ci][:],
                                        lhsT,
                                        w_res[kt][:, off:off + sz],
                                        start=(kt == 0),
                                        stop=(fp8_kt == 0
                                              and kt == bf_kt - 1),
                                    )
                        for j in range(fp8_kt // 2):
                            for si, sub in enumerate(subs):
                                lhsT8 = x8[:, j, :, sub * P:(sub + 1) * P]
                                for ci, (off, sz) in enumerate(CHUNKS):
                                    nc.tensor.matmul(
                                        ps_all[si][ci][:],
                                        lhsT8,
                                        w8_res[j][:, :, off:off + sz],
                                        start=False,
                                        stop=(j == fp8_kt // 2 - 1),
                                        perf_mode=(
                                            mybir.MatmulPerfMode.DoubleRow
                                        ),
                                    )
                        for si, sub in enumerate(subs):
                            drain(sub, tok0, ps_all[si])
                    continue
                for sub in range(NSUB):
                    ps = [
                        (pp0 if ci == 0 else pp).tile(
                            [P, sz], f32, tag=f"ps{ci}", name=f"ps{ci}")
                        for ci, (off, sz) in enumerate(CHUNKS)
                    ]
                    # chunk-major on the final sub so chunk drains + output
                    # DMAs overlap the remaining chunks' matmuls (shorter tail)
                    matmuls(ps, xb, x8, sub,
                            chunk_major=(ns == NSUP - 1 and sub == NSUB - 1),
                            dr_first=((ns * NSUB + sub) % 2 == 1))
                    drain(sub, tok0, ps)

    nc.compile()
    return nc


_PROG_CACHE = {}


def _get_program():
    key = "full"
    if key not in _PROG_CACHE:
        _PROG_CACHE[key] = build_program()
    return _PROG_CACHE[key]


def shard_inputs(x, W_q, scale, zero, bias, fp8_kt=FP8_KT, KT=32):
    """Host-side sharding / layout / dtype prep."""
    x = np.asarray(x, dtype=np.float32)
    W_q = np.asarray(W_q)
    scale = np.asarray(scale, dtype=np.float32)
    zero = np.asarray(zero, dtype=np.float32)
    bias = np.asarray(bias, dtype=np.float32)
    bf_kt = KT - fp8_kt

    ntok = x.shape[0] * x.shape[1]
    xt = np.ascontiguousarray(
        x.reshape(ntok, IN_F).T.astype(ml_dtypes.bfloat16)
    )                                                               # [K, NTOK]
    scale_t = np.ascontiguousarray(scale.reshape(R_FULL, IN_F).T)   # [K, R]
    zero_t = np.ascontiguousarray(zero.reshape(R_FULL, IN_F).T)     # [K, R]
    wq_u8 = W_q.astype(np.uint8)                                    # values < 256
    bias_rs = bias.reshape(GROUP, R_FULL)                           # [i, r]

    # [K, OHALF] ib-replicated zero/scale, shared by all cores
    zr = np.ascontiguousarray(
        zero_t[:, None, :].repeat(IB_FULL, 1)).reshape(IN_F, -1)
    sc = np.ascontiguousarray(
        scale_t[:, None, :].repeat(IB_FULL, 1)).reshape(IN_F, -1)
    OH = IB_FULL * R_FULL

    in_maps = []
    for c in range(N_CORES):
        rows = wq_u8[IB_FULL * c: IB_FULL * (c + 1)]                # [4, 704512]
        # [ib, r, k] -> [k, ib, r] -> [K, OHALF]
        hi = np.ascontiguousarray(
            (rows >> 4).reshape(IB_FULL, R_FULL, IN_F).transpose(2, 0, 1)
        ).reshape(IN_F, IB_FULL * R_FULL).astype(np.float32)
        lo = np.ascontiguousarray(
            (rows & 0xF).reshape(IB_FULL, R_FULL, IN_F).transpose(2, 0, 1)
        ).reshape(IN_F, IB_FULL * R_FULL).astype(np.float32)
        wf = np.empty((IN_F, 2 * IB_FULL * R_FULL), dtype=np.float32)
        wf[:, :OH] = (hi - zr) * sc
        wf[:, OH:] = (lo - zr) * sc
        wt = wf.astype(ml_dtypes.bfloat16)

        bias_c = np.concatenate(
            [
                bias_rs[IB_FULL * c: IB_FULL * (c + 1)].ravel(),
                bias_rs[32 + IB_FULL * c: 32 + IB_FULL * (c + 1)].ravel(),
            ]
        )
        m = {"xt": xt, "wt": wt, "bias": bias_c}
        if fp8_kt:
            # fp8 pair tiles cast straight from the f32 dequant (skips the
            # intermediate bf16 rounding of the bf16 path)
            w8 = np.empty((fp8_kt // 2, P, 2, 2 * OH),
                          dtype=ml_dtypes.float8_e4m3)
            for j in range(fp8_kt // 2):
                for sl in range(2):
                    kt = bf_kt + 2 * j + sl
                    w8[j, :, sl, :] = wt[kt * P:(kt + 1) * P, :].astype(
                        ml_dtypes.float8_e4m3)
            m["w8t"] = w8.reshape(fp8_kt // 2, P, 4 * OH)
        in_maps.append(m)
    return in_maps


def gather_output(results, ntok=NTOK_FULL):
    out = np.empty((ntok, OUT_F), dtype=np.float32)
    ohalf = IB_FULL * R_FULL
    for c in range(N_CORES):
        res = results[c]["out"]
        lo = IB_FULL * c * R_FULL
        out[:, lo: lo + ohalf] = res[:, :ohalf]
        lo = (32 + IB_FULL * c) * R_FULL
        out[:, lo: lo + ohalf] = res[:, ohalf:]
    return out


def kernel(x, W_q, scale, zero, bias):
    from concourse.bass_utils import run_bass_kernel_spmd

    x = np.asarray(x)
    b, s, _ = x.shape
    nc = _get_program()
    in_maps = shard_inputs(x, W_q, scale, zero, bias)
    res = run_bass_kernel_spmd(nc, in_maps, list(range(N_CORES)))
    out = gather_output(res.results)
    return out.reshape(b, s, OUT_F)


# revision 40
# speedup vs baseline: 1.0066x; 1.0054x over previous
"""HQQ 4-bit quantized linear layer on 8 Trainium2 NeuronCores.

Reference computation:
    W_r = concat([W_q >> 4, W_q & 0xF], axis=0).astype(f32)    # [64, 704512]
    W   = ((W_r - zero) * scale).reshape(11008, 4096)          # [out, in]
    out = x @ W.T + bias                                        # [4, 2048, 11008]

Group structure: group j = r*4096 + k (r in [0,172), k in [0,4096)) supplies
output feature o = i*172 + r (element i in [0,64) of the group) at input
feature k.  So for a fixed o, every k belongs to a different group, and
W[o, k] = (nib[i, j] - zero[j]) * scale[j] with i = o//172, j = (o%172)*4096+k.

Sharding (column-parallel over output features, SPMD-uniform):
  core c owns W_q byte-rows [4c, 4c+4).  Byte-row b holds the high nibble of
  i=b and the low nibble of i=b+32, so core c produces output features
  o in {(4c+ib)*172 + r} (high) and {(32+4c+ib)*172 + r} (low), ib in [0,4),
  r in [0,172): 1376 contiguous-after-gather features per core.  Every core
  runs the identical program; x / scale / zero are replicated.

Host-side prep: x is transposed and cast to bf16; the per-core weight shard
is dequantized to bf16 [K, 1376] (k-major, exactly the affine the reference
applies, in f32 then rounded once to bf16) and the trailing FP8_KT k-tiles
are additionally pre-cast to fp8e4m3 in the DoubleRow pair-interleaved
layout.  The device streams weights straight into resident SBUF (no on-
device dequant dependency chain, so the prologue is DMA-paced and the PE
starts within a few us).

Device kernel (per core):
  - stream the 32 w k-tiles [128, 1376] bf16 + 3 fp8 pair tiles into
    resident SBUF.
  - stream bf16 x^T tiles [k=128, tokens] (one DMA per 512-token superstep);
    per 128-token sub, matmul-accumulate over k into PSUM (tokens on psum
    partitions): 26 bf16 matmuls + 3 fp8 DoubleRow pair matmuls (2 k-tiles
    per instruction at the bf16 column rate) per 512-out chunk; x fp8 copies
    are cast on ScalarE per superstep.
  - the first superstep runs kt-major in sub-pairs so matmuls consume each
    weight tile as its DMA lands.
  - drain PSUM + bias (broadcast tile) on VectorE, DMA out f32 per chunk.
"""

import os
import sys

for _p in ("/opt/trn_rl_repo",):
    if os.path.isdir(_p) and _p not in sys.path:
        sys.path.insert(0, _p)

import numpy as np
import ml_dtypes

P = 128
IN_F = 4096
OUT_F = 11008
GROUP = 64
R_FULL = 172          # OUT_F // GROUP
IB_FULL = 4           # W_q byte rows per core
N_CORES = 8
NTOK_FULL = 8192      # 4 * 2048

FP8_KT = 6            # trailing k-tiles computed in fp8 DoubleRow pairs


def _chunks(n, maxc=512):
    out = []
    off = 0
    while off < n:
        sz = min(maxc, n - off)
        out.append((off, sz))
        off += sz
    return out


def build_program(KT=32, NSUP=16, SUP=512, IB=IB_FULL, R=R_FULL,
                  num_devices=N_CORES, fp8_kt=FP8_KT, ktmajor_ns=1):
    """Build the SPMD bass program. Returns the compiled Bacc object.

    KT: number of 128-wide k tiles (K = 128*KT)
    NSUP: number of token supersteps;  SUP: tokens per superstep (mult of 128)
    IB: W_q byte rows per core;  R: group minor dim (o = i*R + r)
    fp8_kt: number of trailing k-tiles run as fp8 DoubleRow pairs (even)
    """
    import concourse.bacc as bacc
    import concourse.bass as bass
    import concourse.mybir as mybir
    import concourse.tile as tile
    from concourse.alu_op_type import AluOpType

    f32 = mybir.dt.float32
    bf16 = mybir.dt.bfloat16
    fp8 = mybir.dt.float8e4

    assert fp8_kt % 2 == 0
    bf_kt = KT - fp8_kt

    K = KT * P
    NTOK = NSUP * SUP
    NSUB = SUP // P
    OHALF = IB * R
    OFULL = 2 * OHALF
    CHUNKS = _chunks(OFULL)

    nc = bacc.Bacc(
        "TRN2", target_bir_lowering=False, debug=False, num_devices=num_devices
    )

    xt = nc.dram_tensor("xt", [K, NTOK], bf16, kind="ExternalInput")
    wt = nc.dram_tensor("wt", [K, OFULL], bf16, kind="ExternalInput")
    w8t = (nc.dram_tensor("w8t", [fp8_kt // 2, P, 2 * OFULL], fp8,
                          kind="ExternalInput") if fp8_kt else None)
    bias = nc.dram_tensor("bias", [OFULL], f32, kind="ExternalInput")
    out = nc.dram_tensor("out", [NTOK, OFULL], f32, kind="ExternalOutput")

    with tile.TileContext(nc) as tc:
        with (
            tc.tile_pool(name="cst", bufs=1) as cst,
            tc.tile_pool(name="wres", bufs=1) as wres,
            tc.tile_pool(name="xb", bufs=2) as xbp,
            tc.tile_pool(name="psum", bufs=2, space="PSUM") as pp,
            tc.tile_pool(name="psum0", bufs=3, space="PSUM") as pp0,
            tc.tile_pool(name="psumw", bufs=1, space="PSUM") as ppw,
            tc.tile_pool(name="outp", bufs=2) as op,
        ):
            def load_x(tok0, nsplit=1, xb=None, splits=None):
                if xb is None:
                    xb = xbp.tile([P, KT, SUP], bf16, tag="xb")
                step = KT // nsplit
                for si in (range(nsplit) if splits is None else splits):
                    kt0 = si * step
                    src = bass.AP(
                        xt, kt0 * P * NTOK + tok0,
                        [[NTOK, P], [P * NTOK, step], [1, SUP]],
                    )
                    nc.sync.dma_start(
                        out=xb[:, kt0:kt0 + step, :], in_=src
                    )
                return xb

            # PE warm-up: dummy matmuls on scratch SBUF keep the HAM activity
            # window busy from t~0 so the first real matmuls run at full clock
            warm = cst.tile([P, 640], bf16)
            nc.vector.memset(warm[:], 0)
            psw = ppw.tile([P, 512], f32, tag="psw", name="psw")
            NWARM = 44
            for wi in range(NWARM):
                nc.tensor.matmul(
                    psw[:, :128], warm[:, :P], warm[:, P:128 + P],
                    start=(wi == 0), stop=(wi == NWARM - 1),
                    skip_group_check=True,
                )

            # --- stream the weight shard into resident SBUF ---
            # bf16 tiles only for the bf16 k-range; the fp8 k-tiles live
            # exclusively as w8_res pair tiles
            w_res = [
                wres.tile([P, OFULL], bf16, tag=f"w{kt}", name=f"w{kt}")
                for kt in range(bf_kt)
            ]
            w8_res = [
                wres.tile([P, 2, OFULL], fp8, tag=f"w8_{j}", name=f"w8_{j}")
                for j in range(fp8_kt // 2)
            ]
            # issue order matches first-superstep consumption: x split 0 and
            # the first w tiles first, remaining x splits interleaved just
            # ahead of when their k-tiles are reached, fp8 pair tiles last
            def load_w(kts):
                for kt in kts:
                    nc.sync.dma_start(
                        out=w_res[kt][:], in_=wt[kt * P:(kt + 1) * P, :]
                    )

            xb0 = load_x(0, nsplit=8, splits=[0])
            load_w(range(0, 4))
            load_x(0, nsplit=8, xb=xb0, splits=[1])
            load_w(range(4, 8))
            load_x(0, nsplit=8, xb=xb0, splits=[2])
            load_w(range(8, 12))
            load_x(0, nsplit=8, xb=xb0, splits=[3])
            load_w(range(12, 16))
            load_x(0, nsplit=8, xb=xb0, splits=[4])
            load_w(range(16, 20))
            load_x(0, nsplit=8, xb=xb0, splits=[5])
            load_w(range(20, 24))
            load_x(0, nsplit=8, xb=xb0, splits=[6])
            load_w(range(24, bf_kt))
            load_x(0, nsplit=8, xb=xb0, splits=[7])
            for j in range(fp8_kt // 2):
                nc.sync.dma_start(
                    out=w8_res[j][:].rearrange("p a b -> p (a b)"),
                    in_=w8t[j, :, :],
                )

            # bias broadcast to [128, OFULL] via partition-step-0 DMA read
            bias_b = cst.tile([P, OFULL], f32)
            bias_bcast_src = bass.AP(bias, 0, [[0, P], [1, OFULL]])
            nc.sync.dma_start(out=bias_b[:], in_=bias_bcast_src)

            # --- main GEMM loop ---
            def drain(sub, tok0, ps):
                ot = op.tile([P, OFULL], f32, tag="ot")
                row0 = tok0 + sub * P
                for ci, (off, sz) in enumerate(CHUNKS):
                    nc.vector.tensor_tensor(
                        out=ot[:, off:off + sz], in0=ps[ci][:],
                        in1=bias_b[:, off:off + sz], op=AluOpType.add,
                    )
                    nc.sync.dma_start(
                        out=out[row0:row0 + P, off:off + sz],
                        in_=ot[:, off:off + sz],
                    )

            def cast_x8(xb):
                if not fp8_kt:
                    return None
                x8 = xbp.tile([P, fp8_kt // 2, 2, SUP], fp8, tag="x8")
                for j in range(fp8_kt // 2):
                    for sl in range(2):
                        nc.scalar.copy(
                            out=x8[:, j, sl, :],
                            in_=xb[:, bf_kt + 2 * j + sl, :],
                        )
                return x8

            def matmuls(ps, xb, x8, sub, chunk_major=False, dr_first=False):
                ts = slice(sub * P, (sub + 1) * P)
                # first chunk per unit is the narrow 352-wide one: the
                # per-sub-boundary stall costs ~one fill of the FIRST matmul,
                # so make that matmul the cheapest
                chunk_iter = (
                    [[ci] for ci in range(len(CHUNKS))] if chunk_major
                    else [[len(CHUNKS) - 1] + list(range(len(CHUNKS) - 1))]
                )
                # alternating DR position across subs keeps consecutive subs'
                # DoubleRow blocks adjacent, halving DR<->bf16 mode switches
                bf_units = [("bf", kt) for kt in range(bf_kt)]
                f8_units = [("f8", j) for j in range(fp8_kt // 2)]
                units = (f8_units + bf_units) if dr_first                     else (bf_units + f8_units)
                last = len(units) - 1
                for cis in chunk_iter:
                    for ui, (kind, idx) in enumerate(units):
                        if kind == "bf":
                            lhsT = xb[:, idx, ts]
                            for ci in cis:
                                off, sz = CHUNKS[ci]
                                nc.tensor.matmul(
                                    ps[ci][:],
                                    lhsT,
                                    w_res[idx][:, off:off + sz],
                                    start=(ui == 0),
                                    stop=(ui == last),
                                )
                        else:
                            lhsT8 = x8[:, idx, :, ts]
                            for ci in cis:
                                off, sz = CHUNKS[ci]
                                nc.tensor.matmul(
                                    ps[ci][:],
                                    lhsT8,
                                    w8_res[idx][:, :, off:off + sz],
                                    start=(ui == 0),
                                    stop=(ui == last),
                                    perf_mode=mybir.MatmulPerfMode.DoubleRow,
                                )

            for ns in range(NSUP):
                tok0 = ns * SUP
                xb = xb0 if ns == 0 else load_x(tok0)
                x8 = cast_x8(xb)
                if ns < ktmajor_ns:
                    # kt-major order in sub-pairs: consume each weight tile
                    # across 2 subs x 3 chunks as soon as its DMA lands
                    for sub0 in range(0, NSUB, 2):
                        subs = [sub0, sub0 + 1][:NSUB - sub0]
                        ps_all = [
                            [
                                (pp0 if ci == 0 else pp).tile(
                                    [P, sz], f32, tag=f"ps{ci}",
                                    name=f"ps{ci}")
                                for ci, (off, sz) in enumerate(CHUNKS)
                            ]
                            for _ in subs
                        ]
                        for kt in range(bf_kt):
                            for si, sub in enumerate(subs):
                                lhsT = xb[:, kt, sub * P:(sub + 1) * P]
                                for ci, (off, sz) in enumerate(CHUNKS):
                                    nc.tensor.matmul(
                                        ps_all[si][ci][:],
                                        lhsT,
                                        w_res[kt][:, off:off + sz],
                                        start=(kt == 0),
                                        stop=(fp8_kt == 0
                                              and kt == bf_kt - 1),
                                    )
                        for j in range(fp8_kt // 2):
                            for si, sub in enumerate(subs):
                                lhsT8 = x8[:, j, :, sub * P:(sub + 1) * P]
                                for ci, (off, sz) in enumerate(CHUNKS):
                                    nc.tensor.matmul(
                                        ps_all[si][ci][:],
                                        lhsT8,
                                        w8_res[j][:, :, off:off + sz],
                                        start=False,
                                        stop=(j == fp8_kt // 2 - 1),
                                        perf_mode=(
                                            mybir.MatmulPerfMode.DoubleRow
                                        ),
                                    )
                        for si, sub in enumerate(subs):
                            drain(sub, tok0, ps_all[si])
                    continue
                for sub in range(NSUB):
                    ps = [
                        (pp0 if ci == 0 else pp).tile(
                            [P, sz], f32, tag=f"ps{ci}", name=f"ps{ci}")
                        for ci, (off, sz) in enumerate(CHUNKS)
                    ]
                    # chunk-major on the final sub so chunk drains + output
                    # DMAs overlap the remaining chunks' matmuls (shorter tail)
                    matmuls(ps, xb, x8, sub,
                            chunk_major=(ns == NSUP - 1 and sub == NSUB - 1),
                            dr_first=((ns * NSUB + sub) % 2 == 1))
                    drain(sub, tok0, ps)

    nc.compile()
    return nc


_PROG_CACHE = {}


def _get_program():
    key = "full"
    if key not in _PROG_CACHE:
        _PROG_CACHE[key] = build_program()
    return _PROG_CACHE[key]


def shard_inputs(x, W_q, scale, zero, bias, fp8_kt=FP8_KT, KT=32):
    """Host-side sharding / layout / dtype prep."""
    x = np.asarray(x, dtype=np.float32)
    W_q = np.asarray(W_q)
    scale = np.asarray(scale, dtype=np.float32)
    zero = np.asarray(zero, dtype=np.float32)
    bias = np.asarray(bias, dtype=np.float32)
    bf_kt = KT - fp8_kt

    ntok = x.shape[0] * x.shape[1]
    xt = np.ascontiguousarray(
        x.reshape(ntok, IN_F).T.astype(ml_dtypes.bfloat16)
    )                                                               # [K, NTOK]
    scale_t = np.ascontiguousarray(scale.reshape(R_FULL, IN_F).T)   # [K, R]
    zero_t = np.ascontiguousarray(zero.reshape(R_FULL, IN_F).T)     # [K, R]
    wq_u8 = W_q.astype(np.uint8)                                    # values < 256
    bias_rs = bias.reshape(GROUP, R_FULL)                           # [i, r]

    # [K, OHALF] ib-replicated zero/scale, shared by all cores
    zr = np.ascontiguousarray(
        zero_t[:, None, :].repeat(IB_FULL, 1)).reshape(IN_F, -1)
    sc = np.ascontiguousarray(
        scale_t[:, None, :].repeat(IB_FULL, 1)).reshape(IN_F, -1)
    OH = IB_FULL * R_FULL

    in_maps = []
    for c in range(N_CORES):
        rows = wq_u8[IB_FULL * c: IB_FULL * (c + 1)]                # [4, 704512]
        # [ib, r, k] -> [k, ib, r] -> [K, OHALF]
        hi = np.ascontiguousarray(
            (rows >> 4).reshape(IB_FULL, R_FULL, IN_F).transpose(2, 0, 1)
        ).reshape(IN_F, IB_FULL * R_FULL).astype(np.float32)
        lo = np.ascontiguousarray(
            (rows & 0xF).reshape(IB_FULL, R_FULL, IN_F).transpose(2, 0, 1)
        ).reshape(IN_F, IB_FULL * R_FULL).astype(np.float32)
        wf = np.empty((IN_F, 2 * IB_FULL * R_FULL), dtype=np.float32)
        wf[:, :OH] = (hi - zr) * sc
        wf[:, OH:] = (lo - zr) * sc
        wt = wf.astype(ml_dtypes.bfloat16)

        bias_c = np.concatenate(
            [
                bias_rs[IB_FULL * c: IB_FULL * (c + 1)].ravel(),
                bias_rs[32 + IB_FULL * c: 32 + IB_FULL * (c + 1)].ravel(),
            ]
        )
        m = {"xt": xt, "wt": wt, "bias": bias_c}
        if fp8_kt:
            # fp8 pair tiles cast straight from the f32 dequant (skips the
            # intermediate bf16 rounding of the bf16 path)
            w8 = np.empty((fp8_kt // 2, P, 2, 2 * OH),
                          dtype=ml_dtypes.float8_e4m3)
            for j in range(fp8_kt // 2):
                for sl in range(2):
                    kt = bf_kt + 2 * j + sl
                    w8[j, :, sl, :] = wt[kt * P:(kt + 1) * P, :].astype(
                        ml_dtypes.float8_e4m3)
            m["w8t"] = w8.reshape(fp8_kt // 2, P, 4 * OH)
        in_maps.append(m)
    return in_maps


def gather_output(results, ntok=NTOK_FULL):
    out = np.empty((ntok, OUT_F), dtype=np.float32)
    ohalf = IB_FULL * R_FULL
    for c in range(N_CORES):
        res = results[c]["out"]
        lo = IB_FULL * c * R_FULL
        out[:, lo: lo + ohalf] = res[:, :ohalf]
        lo = (32 + IB_FULL * c) * R_FULL
        out[:, lo: lo + ohalf] = res[:, ohalf:]
    return out


def kernel(x, W_q, scale, zero, bias):
    from concourse.bass_utils import run_bass_kernel_spmd

    x = np.asarray(x)
    b, s, _ = x.shape
    nc = _get_program()
    in_maps = shard_inputs(x, W_q, scale, zero, bias)
    res = run_bass_kernel_spmd(nc, in_maps, list(range(N_CORES)))
    out = gather_output(res.results)
    return out.reshape(b, s, OUT_F)
